# revision 12
# baseline (speedup 1.0000x reference)
"""Trainium2 Bass kernel: per-row InstanceNorm + Linear(512->512) + ReLU.

Computes, for x [N, 512], W [512, 512], b [512]:
    xn = (x - mean_row) * rsqrt(var_row + 1e-5)      (biased var, per row)
    y  = relu(xn @ W.T + b)

Strategy (v2): data-parallel over rows across 8 NeuronCores, bf16 I/O.
Host-side preprocessing (layout/dtype only, plus tiny weight algebra):
  - x cast to bf16 (halves input DMA).
  - Wc = W - rowmean(W): then (x*rstd) @ Wc.T == ((x-mean)*rstd) @ W.T
    exactly, so the device never needs the row means.
  - output returned transposed (yT [512, rows]) in bf16; host transposes
    back and casts to fp32.

Device per 128-row tile (4 tiles = one 512-row supertile):
  bn_stats (grouped, one instr per supertile) -> bn_aggr -> var
  -> ACT sqrt(var+eps) -> DVE reciprocal -> rstd
  -> D = I * rstd  (DVE tensor_scalar, bf16 diag matrix)
  -> 4x PE "scaled transpose": regular matmul x_chunk.T @ D -> xsT (psum)
  -> ACT evac psum->sbuf (bf16)
  -> 16 matmuls per supertile vs host-prepped WcT -> yT psum [o,128 x n,512]
  -> evac with fused relu+bias (per-partition in yT layout), split ACT/DVE
  -> DMA out yT bf16.
"""

import os
import sys

import numpy as np

sys.path.insert(0, "/opt/trn_rl_repo")

import ml_dtypes  # noqa: E402

import concourse.bacc as bacc  # noqa: E402
import concourse.bass as bass  # noqa: E402
import concourse.tile as tile  # noqa: E402
from concourse import mybir  # noqa: E402
from concourse.bass_utils import run_bass_kernel_spmd  # noqa: E402

N_CORES = 8
N_FULL = 200000
N_IN = 512
N_OUT = 512
P = 128
KC = N_IN // P  # 4 contraction chunks
OC = N_OUT // P  # 4 output chunks
ST = 4  # row-tiles per supertile (512 rows)
ROWS_PER_CORE = 25088  # 49 supertiles of 512; 8*25088 = 200704 >= 200000
N_PAD = ROWS_PER_CORE * N_CORES

EPS = 1e-5

F32 = mybir.dt.float32
BF16 = mybir.dt.bfloat16

# Tuning knobs (env-overridable for quick A/B on hardware)
STATS_DT = BF16 if os.environ.get("K_STATS_BF16", "1") == "1" else F32
# How many of the 4 yT-output evacuations go to the DVE (rest on ACT)
YT_EVAC_DVE = int(os.environ.get("K_YT_DVE", "2"))
# How many of the 4 xsT-transpose evacuations go to the DVE (rest on ACT)
XS_EVAC_DVE = int(os.environ.get("K_XS_DVE", "0"))

LAST_RUN = None  # BassKernelResults of the most recent run (for test harness)


def build_bass(rows_per_core: int) -> bass.Bass:
    rows_per_st = P * ST
    nst = rows_per_core // rows_per_st
    assert rows_per_core % rows_per_st == 0

    nc = bacc.Bacc()
    x_d = nc.declare_dram_parameter("x", [rows_per_core, N_IN], BF16, isOutput=False)
    wt_d = nc.declare_dram_parameter("wt", [N_IN, N_OUT], BF16, isOutput=False)
    b_d = nc.declare_dram_parameter("bvec", [P, OC], F32, isOutput=False)
    ident_d = nc.declare_dram_parameter("ident", [P, P], BF16, isOutput=False)
    yt_d = nc.declare_dram_parameter("yt", [N_OUT, rows_per_core], BF16, isOutput=True)

    with tile.TileContext(nc) as tc:
        with (
            tc.tile_pool(name="singles", bufs=1) as singles,
            tc.tile_pool(name="xin", bufs=3) as xin_pool,
            tc.tile_pool(name="stats", bufs=4) as stats_pool,
            tc.tile_pool(name="dmat", bufs=6) as d_pool,
            tc.tile_pool(name="xnt", bufs=3) as xnt_pool,
            tc.tile_pool(name="yout", bufs=3) as y_pool,
            tc.tile_pool(name="pst", bufs=3, space="PSUM") as pst_pool,
            tc.tile_pool(name="psy", bufs=5, space="PSUM") as psy_pool,
        ):
            # --- constants (loaded once) ---
            # wt_sb[p, c, o] = Wc.T[c*128+p, o]
            wt_sb = singles.tile([P, KC, N_OUT], BF16)
            nc.sync.dma_start(out=wt_sb, in_=wt_d[:, :].rearrange("(c p) o -> p c o", p=P))
            ident_sb = singles.tile([P, P], BF16)
            nc.sync.dma_start(out=ident_sb, in_=ident_d[:, :])
            bias_sb = singles.tile([P, OC], F32)  # bias_sb[p, oc] = b[oc*128+p]
            nc.sync.dma_start(out=bias_sb, in_=b_d[:, :])
            eps_sb = singles.tile([P, 1], F32)
            nc.vector.memset(eps_sb, EPS)

            # supertile s, partition p, sub-tile j  <->  row s*512 + p*ST + j
            x_b = x_d[:, :].rearrange("(s p j) i -> s p j i", p=P, j=ST)
            # yt_d[o, col]: col = s*512 + n; pair supertiles so each output
            # descriptor covers 2 adjacent 512-col blocks (2KB runs).
            y_flat = yt_d[:, :].rearrange("(oc p) col -> p oc col", p=P)

            PST = P * ST
            groups = [(g, min(2, nst - g)) for g in range(0, nst, 2)]
            for g, gsz in groups:
                yb = y_pool.tile([P, OC, 2 * PST], BF16)
                for k in range(gsz):
                    s = g + k
                    xb = xin_pool.tile([P, ST, N_IN], BF16)
                    nc.sync.dma_start(out=xb, in_=x_b[s])

                    # --- stats: per-tile bn_stats (FMAX=512) ---
                    st6 = stats_pool.tile([P, ST, 6], STATS_DT)
                    for j in range(ST):
                        nc.vector.bn_stats(out=st6[:, j, :], in_=xb[:, j, :])
                    mv = stats_pool.tile([P, ST, 2], F32)
                    for j in range(ST):
                        nc.vector.bn_aggr(out=mv[:, j, :], in_=st6[:, j, :])
                    # sd = sqrt(var + eps) for all ST tiles in one ACT op
                    sd = stats_pool.tile([P, ST], F32)
                    nc.scalar.activation(
                        out=sd, in_=mv[:, :, 1],
                        func=mybir.ActivationFunctionType.Sqrt,
                        bias=eps_sb[:, :], scale=1.0,
                    )
                    rstd = stats_pool.tile([P, ST], F32)
                    nc.vector.reciprocal(out=rstd, in_=sd)

                    # --- scaled transposes: xsT[i, n] = x[n, i] * rstd[n] ---
                    xnt = xnt_pool.tile([P, KC, PST], BF16)  # [i, c, n(4 tiles)]
                    for j in range(ST):
                        dmat = d_pool.tile([P, P], BF16)
                        # D = I * rstd on the (otherwise idle) GPSIMD engine
                        nc.gpsimd.tensor_scalar(
                            out=dmat, in0=ident_sb[:, :],
                            scalar1=rstd[:, j:j + 1], scalar2=None,
                            op0=mybir.AluOpType.mult,
                        )
                        ps_t = pst_pool.tile([P, N_IN], F32)
                        for c in range(KC):
                            nc.tensor.matmul(
                                ps_t[:, c * P:(c + 1) * P],
                                xb[:, j, c * P:(c + 1) * P],  # lhsT [n, i]
                                dmat[:, :],                    # rhs  [n, n]
                                start=True, stop=True,
                            )
                        # evac psum->sbuf: [i, c*128+n'] -> xnt[:, c, j*128+n']
                        if j < XS_EVAC_DVE:
                            nc.vector.tensor_copy(
                                xnt[:, :, j * P:(j + 1) * P],
                                ps_t[:, :].rearrange("p (c n) -> p c n", n=P),
                            )
                        else:
                            nc.scalar.copy(
                                xnt[:, :, j * P:(j + 1) * P],
                                ps_t[:, :].rearrange("p (c n) -> p c n", n=P),
                            )

                    # --- main matmuls: yT[oc] = WcT[:,oc].T @ xsT (+bias, relu) ---
                    for oc in range(OC):
                        ps_y = psy_pool.tile([P, PST], F32)
                        for c in range(KC):
                            nc.tensor.matmul(
                                ps_y[:, :],
                                wt_sb[:, c, oc * P:(oc + 1) * P],  # lhsT [i, o]
                                xnt[:, c, :],                       # rhs  [i, n]
                                start=(c == 0),
                                stop=(c == KC - 1),
                            )
                        yslice = yb[:, oc, k * PST:(k + 1) * PST]
                        # evac with fused relu + per-partition bias; split ACT/DVE
                        if oc < YT_EVAC_DVE:
                            nc.vector.tensor_scalar(
                                out=yslice, in0=ps_y[:, :],
                                scalar1=bias_sb[:, oc:oc + 1], scalar2=0.0,
                                op0=mybir.AluOpType.add, op1=mybir.AluOpType.max,
                            )
                        else:
                            nc.scalar.activation(
                                out=yslice, in_=ps_y[:, :],
                                func=mybir.ActivationFunctionType.Relu,
                                bias=bias_sb[:, oc:oc + 1], scale=1.0,
                            )
                nc.sync.dma_start(
                    out=y_flat[:, :, g * PST:(g + gsz) * PST],
                    in_=yb[:, :, :gsz * PST],
                )
    nc.compile()
    return nc


_BASS_CACHE: dict[int, bass.Bass] = {}


def _get_bass(rows_per_core: int) -> bass.Bass:
    if rows_per_core not in _BASS_CACHE:
        _BASS_CACHE[rows_per_core] = build_bass(rows_per_core)
    return _BASS_CACHE[rows_per_core]


def _run(x_pad: np.ndarray, W: np.ndarray, b: np.ndarray, rows_per_core: int) -> np.ndarray:
    """x_pad: [n_cores*rows_per_core, 512] bf16. Returns [n_cores*rows, 512] f32."""
    global LAST_RUN
    nc = _get_bass(rows_per_core)
    # center W rows so the matmul on (x*rstd) implements the mean subtraction
    Wc = W - W.mean(axis=1, keepdims=True)
    wt = np.ascontiguousarray(Wc.T).astype(ml_dtypes.bfloat16)
    bb = np.ascontiguousarray(b.reshape(OC, P).T).astype(np.float32)  # [P, OC]
    ident = np.eye(P, dtype=ml_dtypes.bfloat16)
    in_maps = [
        {
            "x": np.ascontiguousarray(x_pad[c * rows_per_core:(c + 1) * rows_per_core]),
            "wt": wt,
            "bvec": bb,
            "ident": ident,
        }
        for c in range(N_CORES)
    ]
    trace = bool(os.environ.get("BASS_TRACE"))
    res = run_bass_kernel_spmd(nc, in_maps, list(range(N_CORES)), trace=trace)
    LAST_RUN = res
    # yt: [512, rows_per_core] bf16 per core. Device column s*512 + j*128 + p
    # holds row s*512 + p*ST + j (interleaved DMA layout): unpermute, then
    # transpose to [rows, 512] and cast to f32.
    nst = rows_per_core // (P * ST)
    outs = []
    for c in range(N_CORES):
        yt = np.asarray(res.results[c]["yt"])  # [512, rows] bf16
        y = yt.reshape(N_OUT, nst, ST, P).transpose(1, 3, 2, 0)  # [s, p, j, o]
        outs.append(y.reshape(rows_per_core, N_OUT).astype(np.float32))
    return np.concatenate(outs, axis=0)


def kernel(x: np.ndarray, W: np.ndarray, b: np.ndarray) -> np.ndarray:
    x = np.asarray(x, dtype=np.float32)
    W = np.asarray(W, dtype=np.float32)
    b = np.asarray(b, dtype=np.float32)
    n = x.shape[0]
    x_pad = np.zeros((N_PAD, N_IN), dtype=ml_dtypes.bfloat16)
    x_pad[:n] = x.astype(ml_dtypes.bfloat16)
    y_pad = _run(x_pad, W, b, ROWS_PER_CORE)
    return np.ascontiguousarray(y_pad[:n])


# revision 14
# speedup vs baseline: 1.3511x; 1.3511x over previous
"""Trainium2 Bass kernel: per-row InstanceNorm + Linear(512->512) + ReLU.

Computes, for x [N, 512], W [512, 512], b [512]:
    xn = (x - mean_row) * rsqrt(var_row + 1e-5)      (biased var, per row)
    y  = relu(xn @ W.T + b)

Strategy (v2): data-parallel over rows across 8 NeuronCores, bf16 I/O.
Host-side preprocessing (layout/dtype only, plus tiny weight algebra):
  - x cast to bf16 (halves input DMA).
  - Wc = W - rowmean(W): then (x*rstd) @ Wc.T == ((x-mean)*rstd) @ W.T
    exactly, so the device never needs the row means.
  - output returned transposed (yT [512, rows]) in bf16; host transposes
    back and casts to fp32.

Device per 128-row tile (4 tiles = one 512-row supertile):
  bn_stats (grouped, one instr per supertile) -> bn_aggr -> var
  -> ACT sqrt(var+eps) -> DVE reciprocal -> rstd
  -> D = I * rstd  (DVE tensor_scalar, bf16 diag matrix)
  -> 4x PE "scaled transpose": regular matmul x_chunk.T @ D -> xsT (psum)
  -> ACT evac psum->sbuf (bf16)
  -> 16 matmuls per supertile vs host-prepped WcT -> yT psum [o,128 x n,512]
  -> evac with fused relu+bias (per-partition in yT layout), split ACT/DVE
  -> DMA out yT bf16.
"""

import os
import sys

import numpy as np

sys.path.insert(0, "/opt/trn_rl_repo")

import ml_dtypes  # noqa: E402

import concourse.bacc as bacc  # noqa: E402
import concourse.bass as bass  # noqa: E402
import concourse.tile as tile  # noqa: E402
from concourse import mybir  # noqa: E402
from concourse.bass_utils import run_bass_kernel_spmd  # noqa: E402

N_CORES = 8
N_FULL = 200000
N_IN = 512
N_OUT = 512
P = 128
KC = N_IN // P  # 4 contraction chunks
OC = N_OUT // P  # 4 output chunks
ST = 4  # row-tiles per supertile (512 rows)
ROWS_PER_CORE = 25088  # 49 supertiles of 512; 8*25088 = 200704 >= 200000
N_PAD = ROWS_PER_CORE * N_CORES

EPS = 1e-5

F32 = mybir.dt.float32
BF16 = mybir.dt.bfloat16

# Tuning knobs (env-overridable for quick A/B on hardware)
STATS_DT = BF16 if os.environ.get("K_STATS_BF16", "1") == "1" else F32
# How many of the 4 yT-output evacuations go to the DVE (rest on ACT)
YT_EVAC_DVE = int(os.environ.get("K_YT_DVE", "2"))
# How many of the 4 xsT-transpose evacuations go to the DVE (rest on ACT)
XS_EVAC_DVE = int(os.environ.get("K_XS_DVE", "0"))

LAST_RUN = None  # BassKernelResults of the most recent run (for test harness)


def build_bass(rows_per_core: int) -> bass.Bass:
    rows_per_st = P * ST
    nst = rows_per_core // rows_per_st
    assert rows_per_core % rows_per_st == 0

    nc = bacc.Bacc()
    x_d = nc.declare_dram_parameter("x", [rows_per_core, N_IN], BF16, isOutput=False)
    wt_d = nc.declare_dram_parameter("wt", [N_IN, N_OUT], BF16, isOutput=False)
    b_d = nc.declare_dram_parameter("bvec", [P, OC], F32, isOutput=False)
    ident_d = nc.declare_dram_parameter("ident", [P, P], BF16, isOutput=False)
    yt_d = nc.declare_dram_parameter("yt", [N_OUT, rows_per_core], BF16, isOutput=True)

    with tile.TileContext(nc) as tc:
        with (
            tc.tile_pool(name="singles", bufs=1) as singles,
            tc.tile_pool(name="xin", bufs=3) as xin_pool,
            tc.tile_pool(name="stats", bufs=4) as stats_pool,
            tc.tile_pool(name="dmat", bufs=6) as d_pool,
            tc.tile_pool(name="xnt", bufs=3) as xnt_pool,
            tc.tile_pool(name="yout", bufs=3) as y_pool,
            tc.tile_pool(name="pst", bufs=3, space="PSUM") as pst_pool,
            tc.tile_pool(name="psy", bufs=5, space="PSUM") as psy_pool,
        ):
            # --- constants (loaded once) ---
            # wt_sb[p, c, o] = Wc.T[c*128+p, o]
            wt_sb = singles.tile([P, KC, N_OUT], BF16)
            nc.sync.dma_start(out=wt_sb, in_=wt_d[:, :].rearrange("(c p) o -> p c o", p=P))
            ident_sb = singles.tile([P, P], BF16)
            nc.sync.dma_start(out=ident_sb, in_=ident_d[:, :])
            bias_sb = singles.tile([P, OC], F32)  # bias_sb[p, oc] = b[oc*128+p]
            nc.sync.dma_start(out=bias_sb, in_=b_d[:, :])
            eps_sb = singles.tile([P, 1], F32)
            nc.vector.memset(eps_sb, EPS)

            # supertile s, partition p, sub-tile j  <->  row s*512 + p*ST + j
            x_b = x_d[:, :].rearrange("(s p j) i -> s p j i", p=P, j=ST)
            # yt_d[o, col]: col = s*512 + n; pair supertiles so each output
            # descriptor covers 2 adjacent 512-col blocks (2KB runs).
            y_flat = yt_d[:, :].rearrange("(oc p) col -> p oc col", p=P)

            PST = P * ST
            groups = [(g, min(2, nst - g)) for g in range(0, nst, 2)]
            for g, gsz in groups:
                yb = y_pool.tile([P, OC, 2 * PST], BF16)
                for k in range(gsz):
                    s = g + k
                    xb = xin_pool.tile([P, ST, N_IN], BF16)
                    nc.sync.dma_start(out=xb, in_=x_b[s])

                    # --- stats: per-tile bn_stats (FMAX=512) ---
                    st6 = stats_pool.tile([P, ST, 6], STATS_DT)
                    for j in range(ST):
                        nc.vector.bn_stats(out=st6[:, j, :], in_=xb[:, j, :])
                    mv = stats_pool.tile([P, ST, 2], F32)
                    for j in range(ST):
                        nc.vector.bn_aggr(out=mv[:, j, :], in_=st6[:, j, :])
                    # sd = sqrt(var + eps) for all ST tiles in one ACT op
                    sd = stats_pool.tile([P, ST], F32)
                    nc.scalar.activation(
                        out=sd, in_=mv[:, :, 1],
                        func=mybir.ActivationFunctionType.Sqrt,
                        bias=eps_sb[:, :], scale=1.0,
                    )
                    rstd = stats_pool.tile([P, ST], F32)
                    nc.vector.reciprocal(out=rstd, in_=sd)

                    # --- scale rows, then plain bf16 transposes (bf16 PSUM) ---
                    xnt = xnt_pool.tile([P, KC, PST], BF16)  # [i, c, n(4 tiles)]
                    for j in range(ST):
                        xs = d_pool.tile([P, N_IN], BF16)
                        nc.vector.tensor_scalar(
                            out=xs, in0=xb[:, j, :],
                            scalar1=rstd[:, j:j + 1], scalar2=None,
                            op0=mybir.AluOpType.mult,
                        )
                        ps_t = pst_pool.tile([P, N_IN], BF16)
                        for c in range(KC):
                            nc.tensor.transpose(
                                ps_t[:, c * P:(c + 1) * P],
                                xs[:, c * P:(c + 1) * P],
                                ident_sb[:, :],
                            )
                        # evac psum->sbuf: [i, c*128+n'] -> xnt[:, c, j*128+n']
                        if j < XS_EVAC_DVE:
                            nc.vector.tensor_copy(
                                xnt[:, :, j * P:(j + 1) * P],
                                ps_t[:, :].rearrange("p (c n) -> p c n", n=P),
                            )
                        else:
                            nc.scalar.copy(
                                xnt[:, :, j * P:(j + 1) * P],
                                ps_t[:, :].rearrange("p (c n) -> p c n", n=P),
                            )

                    # --- main matmuls: yT[oc] = WcT[:,oc].T @ xsT (+bias, relu) ---
                    for oc in range(OC):
                        ps_y = psy_pool.tile([P, PST], F32)
                        for c in range(KC):
                            nc.tensor.matmul(
                                ps_y[:, :],
                                wt_sb[:, c, oc * P:(oc + 1) * P],  # lhsT [i, o]
                                xnt[:, c, :],                       # rhs  [i, n]
                                start=(c == 0),
                                stop=(c == KC - 1),
                            )
                        yslice = yb[:, oc, k * PST:(k + 1) * PST]
                        # evac with fused relu + per-partition bias; split ACT/DVE
                        # (alternate 2/2 and 1/3 across supertiles -> avg 1.5 DVE)
                        n_dve = YT_EVAC_DVE if s % 2 == 0 else max(YT_EVAC_DVE - 1, 0)
                        if oc < n_dve:
                            nc.vector.tensor_scalar(
                                out=yslice, in0=ps_y[:, :],
                                scalar1=bias_sb[:, oc:oc + 1], scalar2=0.0,
                                op0=mybir.AluOpType.add, op1=mybir.AluOpType.max,
                            )
                        else:
                            nc.scalar.activation(
                                out=yslice, in_=ps_y[:, :],
                                func=mybir.ActivationFunctionType.Relu,
                                bias=bias_sb[:, oc:oc + 1], scale=1.0,
                            )
                nc.sync.dma_start(
                    out=y_flat[:, :, g * PST:(g + gsz) * PST],
                    in_=yb[:, :, :gsz * PST],
                )
    nc.compile()
    return nc


_BASS_CACHE: dict[int, bass.Bass] = {}


def _get_bass(rows_per_core: int) -> bass.Bass:
    if rows_per_core not in _BASS_CACHE:
        _BASS_CACHE[rows_per_core] = build_bass(rows_per_core)
    return _BASS_CACHE[rows_per_core]


def _run(x_pad: np.ndarray, W: np.ndarray, b: np.ndarray, rows_per_core: int) -> np.ndarray:
    """x_pad: [n_cores*rows_per_core, 512] bf16. Returns [n_cores*rows, 512] f32."""
    global LAST_RUN
    nc = _get_bass(rows_per_core)
    # center W rows so the matmul on (x*rstd) implements the mean subtraction
    Wc = W - W.mean(axis=1, keepdims=True)
    wt = np.ascontiguousarray(Wc.T).astype(ml_dtypes.bfloat16)
    bb = np.ascontiguousarray(b.reshape(OC, P).T).astype(np.float32)  # [P, OC]
    ident = np.eye(P, dtype=ml_dtypes.bfloat16)
    in_maps = [
        {
            "x": np.ascontiguousarray(x_pad[c * rows_per_core:(c + 1) * rows_per_core]),
            "wt": wt,
            "bvec": bb,
            "ident": ident,
        }
        for c in range(N_CORES)
    ]
    trace = bool(os.environ.get("BASS_TRACE"))
    res = run_bass_kernel_spmd(nc, in_maps, list(range(N_CORES)), trace=trace)
    LAST_RUN = res
    # yt: [512, rows_per_core] bf16 per core. Device column s*512 + j*128 + p
    # holds row s*512 + p*ST + j (interleaved DMA layout): unpermute, then
    # transpose to [rows, 512] and cast to f32.
    nst = rows_per_core // (P * ST)
    outs = []
    for c in range(N_CORES):
        yt = np.asarray(res.results[c]["yt"])  # [512, rows] bf16
        y = yt.reshape(N_OUT, nst, ST, P).transpose(1, 3, 2, 0)  # [s, p, j, o]
        outs.append(y.reshape(rows_per_core, N_OUT).astype(np.float32))
    return np.concatenate(outs, axis=0)


def kernel(x: np.ndarray, W: np.ndarray, b: np.ndarray) -> np.ndarray:
    x = np.asarray(x, dtype=np.float32)
    W = np.asarray(W, dtype=np.float32)
    b = np.asarray(b, dtype=np.float32)
    n = x.shape[0]
    x_pad = np.zeros((N_PAD, N_IN), dtype=ml_dtypes.bfloat16)
    x_pad[:n] = x.astype(ml_dtypes.bfloat16)
    y_pad = _run(x_pad, W, b, ROWS_PER_CORE)
    return np.ascontiguousarray(y_pad[:n])


# revision 16
# speedup vs baseline: 1.4287x; 1.0574x over previous
"""Trainium2 Bass kernel: per-row InstanceNorm + Linear(512->512) + ReLU.

Computes, for x [N, 512], W [512, 512], b [512]:
    xn = (x - mean_row) * rsqrt(var_row + 1e-5)      (biased var, per row)
    y  = relu(xn @ W.T + b)

Strategy (v2): data-parallel over rows across 8 NeuronCores, bf16 I/O.
Host-side preprocessing (layout/dtype only, plus tiny weight algebra):
  - x cast to bf16 (halves input DMA).
  - Wc = W - rowmean(W): then (x*rstd) @ Wc.T == ((x-mean)*rstd) @ W.T
    exactly, so the device never needs the row means.
  - output returned transposed (yT [512, rows]) in bf16; host transposes
    back and casts to fp32.

Device per 128-row tile (4 tiles = one 512-row supertile):
  bn_stats (grouped, one instr per supertile) -> bn_aggr -> var
  -> ACT sqrt(var+eps) -> DVE reciprocal -> rstd
  -> D = I * rstd  (DVE tensor_scalar, bf16 diag matrix)
  -> 4x PE "scaled transpose": regular matmul x_chunk.T @ D -> xsT (psum)
  -> ACT evac psum->sbuf (bf16)
  -> 16 matmuls per supertile vs host-prepped WcT -> yT psum [o,128 x n,512]
  -> evac with fused relu+bias (per-partition in yT layout), split ACT/DVE
  -> DMA out yT bf16.
"""

import os
import sys

import numpy as np

sys.path.insert(0, "/opt/trn_rl_repo")

import ml_dtypes  # noqa: E402

import concourse.bacc as bacc  # noqa: E402
import concourse.bass as bass  # noqa: E402
import concourse.tile as tile  # noqa: E402
from concourse import mybir  # noqa: E402
from concourse.bass_utils import run_bass_kernel_spmd  # noqa: E402

N_CORES = 8
N_FULL = 200000
N_IN = 512
N_OUT = 512
P = 128
KC = N_IN // P  # 4 contraction chunks
OC = N_OUT // P  # 4 output chunks
ST = 4  # row-tiles per supertile (512 rows)
ROWS_PER_CORE = 25088  # 49 supertiles of 512; 8*25088 = 200704 >= 200000
N_PAD = ROWS_PER_CORE * N_CORES

EPS = 1e-5

F32 = mybir.dt.float32
BF16 = mybir.dt.bfloat16

# Tuning knobs (env-overridable for quick A/B on hardware)
STATS_DT = BF16 if os.environ.get("K_STATS_BF16", "1") == "1" else F32
# How many of the 4 yT-output evacuations go to the DVE (rest on ACT)
YT_EVAC_DVE = int(os.environ.get("K_YT_DVE", "2"))
# How many of the 4 xsT-transpose evacuations go to the DVE (rest on ACT)
XS_EVAC_DVE = int(os.environ.get("K_XS_DVE", "0"))

LAST_RUN = None  # BassKernelResults of the most recent run (for test harness)


def build_bass(rows_per_core: int) -> bass.Bass:
    rows_per_st = P * ST
    nst = rows_per_core // rows_per_st
    assert rows_per_core % rows_per_st == 0

    nc = bacc.Bacc()
    x_d = nc.declare_dram_parameter("x", [rows_per_core, N_IN], BF16, isOutput=False)
    wt_d = nc.declare_dram_parameter("wt", [N_IN, N_OUT], BF16, isOutput=False)
    b_d = nc.declare_dram_parameter("bvec", [P, OC], F32, isOutput=False)
    ident_d = nc.declare_dram_parameter("ident", [P, P], BF16, isOutput=False)
    yt_d = nc.declare_dram_parameter("yt", [N_OUT, rows_per_core], BF16, isOutput=True)

    with tile.TileContext(nc) as tc:
        with (
            tc.tile_pool(name="singles", bufs=1) as singles,
            tc.tile_pool(name="xin", bufs=3) as xin_pool,
            tc.tile_pool(name="stats", bufs=12) as stats_pool,
            tc.tile_pool(name="dmat", bufs=6) as d_pool,
            tc.tile_pool(name="xnt", bufs=3) as xnt_pool,
            tc.tile_pool(name="yout", bufs=3) as y_pool,
            tc.tile_pool(name="pst", bufs=3, space="PSUM") as pst_pool,
            tc.tile_pool(name="psy", bufs=5, space="PSUM") as psy_pool,
        ):
            # --- constants (loaded once) ---
            # wt_sb[p, c, o] = Wc.T[c*128+p, o]
            wt_sb = singles.tile([P, KC, N_OUT], BF16)
            nc.sync.dma_start(out=wt_sb, in_=wt_d[:, :].rearrange("(c p) o -> p c o", p=P))
            ident_sb = singles.tile([P, P], BF16)
            nc.sync.dma_start(out=ident_sb, in_=ident_d[:, :])
            bias_sb = singles.tile([P, OC], F32)  # bias_sb[p, oc] = b[oc*128+p]
            nc.sync.dma_start(out=bias_sb, in_=b_d[:, :])
            eps_sb = singles.tile([P, 1], F32)
            nc.vector.memset(eps_sb, EPS)

            # supertile s, partition p, sub-tile j  <->  row s*512 + p*ST + j
            x_b = x_d[:, :].rearrange("(s p j) i -> s p j i", p=P, j=ST)
            # yt_d[o, col]: col = s*512 + n; pair supertiles so each output
            # descriptor covers 2 adjacent 512-col blocks (2KB runs).
            y_flat = yt_d[:, :].rearrange("(oc p) col -> p oc col", p=P)

            PST = P * ST
            groups = [(g, min(2, nst - g)) for g in range(0, nst, 2)]
            for g, gsz in groups:
                yb = y_pool.tile([P, OC, 2 * PST], BF16)
                for k in range(gsz):
                    s = g + k
                    xb = xin_pool.tile([P, ST, N_IN], BF16)
                    nc.sync.dma_start(out=xb, in_=x_b[s])

                    # --- stats: per-tile bn_stats (FMAX=512) ---
                    st6 = stats_pool.tile([P, ST, 6], STATS_DT)
                    for j in range(ST):
                        nc.vector.bn_stats(out=st6[:, j, :], in_=xb[:, j, :])
                    mv = stats_pool.tile([P, ST, 2], F32)
                    for j in range(ST):
                        nc.vector.bn_aggr(out=mv[:, j, :], in_=st6[:, j, :])
                    # sd = sqrt(var + eps) for all ST tiles in one ACT op
                    sd = stats_pool.tile([P, ST], F32)
                    nc.scalar.activation(
                        out=sd, in_=mv[:, :, 1],
                        func=mybir.ActivationFunctionType.Sqrt,
                        bias=eps_sb[:, :], scale=1.0,
                    )
                    rstd = stats_pool.tile([P, ST], F32)
                    nc.vector.reciprocal(out=rstd, in_=sd)

                    # --- scaled transposes: xsT[i, n] = x[n, i] * rstd[n] ---
                    # (regular matmul vs D=I*rstd keeps PE HAM-warm)
                    xnt = xnt_pool.tile([P, KC, PST], BF16)  # [i, c, n(4 tiles)]
                    for j in range(ST):
                        dmat = d_pool.tile([P, P], BF16)
                        nc.vector.tensor_scalar(
                            out=dmat, in0=ident_sb[:, :],
                            scalar1=rstd[:, j:j + 1], scalar2=None,
                            op0=mybir.AluOpType.mult,
                        )
                        ps_t = pst_pool.tile([P, N_IN], F32)
                        for c in range(KC):
                            nc.tensor.matmul(
                                ps_t[:, c * P:(c + 1) * P],
                                xb[:, j, c * P:(c + 1) * P],  # lhsT [n, i]
                                dmat[:, :],                    # rhs  [n, n]
                                start=True, stop=True,
                            )
                        # evac psum->sbuf: [i, c*128+n'] -> xnt[:, c, j*128+n']
                        if j < XS_EVAC_DVE:
                            nc.vector.tensor_copy(
                                xnt[:, :, j * P:(j + 1) * P],
                                ps_t[:, :].rearrange("p (c n) -> p c n", n=P),
                            )
                        else:
                            nc.scalar.copy(
                                xnt[:, :, j * P:(j + 1) * P],
                                ps_t[:, :].rearrange("p (c n) -> p c n", n=P),
                            )

                    # --- main matmuls: yT[oc] = WcT[:,oc].T @ xsT (+bias, relu) ---
                    for oc in range(OC):
                        ps_y = psy_pool.tile([P, PST], F32)
                        for c in range(KC):
                            nc.tensor.matmul(
                                ps_y[:, :],
                                wt_sb[:, c, oc * P:(oc + 1) * P],  # lhsT [i, o]
                                xnt[:, c, :],                       # rhs  [i, n]
                                start=(c == 0),
                                stop=(c == KC - 1),
                            )
                        yslice = yb[:, oc, k * PST:(k + 1) * PST]
                        # evac with fused relu + per-partition bias; split ACT/DVE
                        # (alternate 2/2 and 1/3 across supertiles -> avg 1.5 DVE)
                        n_dve = YT_EVAC_DVE if s % 2 == 0 else max(YT_EVAC_DVE - 1, 0)
                        if oc < n_dve:
                            nc.vector.tensor_scalar(
                                out=yslice, in0=ps_y[:, :],
                                scalar1=bias_sb[:, oc:oc + 1], scalar2=0.0,
                                op0=mybir.AluOpType.add, op1=mybir.AluOpType.max,
                            )
                        else:
                            nc.scalar.activation(
                                out=yslice, in_=ps_y[:, :],
                                func=mybir.ActivationFunctionType.Relu,
                                bias=bias_sb[:, oc:oc + 1], scale=1.0,
                            )
                nc.sync.dma_start(
                    out=y_flat[:, :, g * PST:(g + gsz) * PST],
                    in_=yb[:, :, :gsz * PST],
                )
    nc.compile()
    return nc


_BASS_CACHE: dict[int, bass.Bass] = {}


def _get_bass(rows_per_core: int) -> bass.Bass:
    if rows_per_core not in _BASS_CACHE:
        _BASS_CACHE[rows_per_core] = build_bass(rows_per_core)
    return _BASS_CACHE[rows_per_core]


def _run(x_pad: np.ndarray, W: np.ndarray, b: np.ndarray, rows_per_core: int) -> np.ndarray:
    """x_pad: [n_cores*rows_per_core, 512] bf16. Returns [n_cores*rows, 512] f32."""
    global LAST_RUN
    nc = _get_bass(rows_per_core)
    # center W rows so the matmul on (x*rstd) implements the mean subtraction
    Wc = W - W.mean(axis=1, keepdims=True)
    wt = np.ascontiguousarray(Wc.T).astype(ml_dtypes.bfloat16)
    bb = np.ascontiguousarray(b.reshape(OC, P).T).astype(np.float32)  # [P, OC]
    ident = np.eye(P, dtype=ml_dtypes.bfloat16)
    in_maps = [
        {
            "x": np.ascontiguousarray(x_pad[c * rows_per_core:(c + 1) * rows_per_core]),
            "wt": wt,
            "bvec": bb,
            "ident": ident,
        }
        for c in range(N_CORES)
    ]
    trace = bool(os.environ.get("BASS_TRACE"))
    res = run_bass_kernel_spmd(nc, in_maps, list(range(N_CORES)), trace=trace)
    LAST_RUN = res
    # yt: [512, rows_per_core] bf16 per core. Device column s*512 + j*128 + p
    # holds row s*512 + p*ST + j (interleaved DMA layout): unpermute, then
    # transpose to [rows, 512] and cast to f32.
    nst = rows_per_core // (P * ST)
    outs = []
    for c in range(N_CORES):
        yt = np.asarray(res.results[c]["yt"])  # [512, rows] bf16
        y = yt.reshape(N_OUT, nst, ST, P).transpose(1, 3, 2, 0)  # [s, p, j, o]
        outs.append(y.reshape(rows_per_core, N_OUT).astype(np.float32))
    return np.concatenate(outs, axis=0)


def kernel(x: np.ndarray, W: np.ndarray, b: np.ndarray) -> np.ndarray:
    x = np.asarray(x, dtype=np.float32)
    W = np.asarray(W, dtype=np.float32)
    b = np.asarray(b, dtype=np.float32)
    n = x.shape[0]
    x_pad = np.zeros((N_PAD, N_IN), dtype=ml_dtypes.bfloat16)
    x_pad[:n] = x.astype(ml_dtypes.bfloat16)
    y_pad = _run(x_pad, W, b, ROWS_PER_CORE)
    return np.ascontiguousarray(y_pad[:n])
